# revision 28
# baseline (speedup 1.0000x reference)
"""GAT (3-layer, 4-head) Trainium2 Bass kernel, 8-core SPMD.

Strategy (follows the sharding_hint: graph-partition by dst node, halo
exchange via an AllGather of the per-layer node table, replicated weights):

  - Nodes are partitioned by id across the 8 cores (6250 each).  Edges are
    assigned to the core owning their destination node and sorted by dst.
    Within a core, nodes are packed greedily into dst BLOCKS (<=32 nodes,
    <=512 edges, ~99% slot fill) and each block is padded to exactly 32
    node-SLOTS, so block b always owns slot rows [32b, 32b+32) — a static,
    core-independent layout.  All per-layer node tables (shard / AllGather
    table / xnext) are indexed by slot.  Static bases make the per-block
    s_dst fetch and the per-block output scatter plain strided HWDGE DMAs
    (they were Pool-engine indirect DMAs before — the Pool SWDGE
    descriptor-generation serial time was the kernel's bottleneck).
  - Per layer: a dense phase computes h = x @ W_aug for the core's own slots
    (W_aug's extra columns produce the attention scores s_src/s_dst via
    host-prefolded Wa = W @ a), writes bf16 rows [h | 1 | s_src | s_dst] to a
    local shard, and an AllGather makes the full slot table visible to every
    core (the halo exchange: edges are uniform-random, so every core needs
    nearly every row).
  - Edge phase, per block (4 x 128-edge chunks): one indirect DMA per chunk
    fetches h[src] rows (s_src riding along) into a single per-block tile.
    Per-edge s_dst comes from a tiny matmul s01T^T @ sdh32 (s01T is the
    host-shipped transposed dst-selection matrix, sdh32 the block's 32
    s_dst rows DMA'd from the local shard), giving scores at [edge, head]
    width; leaky-relu + exp run on [128, 16] tiles.  The scatter-add over
    dst is a matmul with S~[e,(head,d)] = exp(score)*S01[e,d] (one wide
    tensor op builds S~); a ones column in the table makes the softmax
    denominator fall out of the same matmul.  Normalized rows go to the
    next layer's slot table with one strided DMA per block (static base).
  - Layer 0 needs no gathers or dense phase at all: out0 =
    (sum_e ex_e x[src_e]) @ W0, with x[src_e] host-pregathered into dense
    per-chunk streams and the numerators ex0 host-computed.
  - Readout: mean/max pooling over graphs runs ON DEVICE (mean as a one-hot
    matmul vs a device-generated selection matrix; max via per-chunk masked
    transposes + free-axis max-reduces, exploiting that a 128-slot chunk
    spans at most 2 graphs).  Each core emits ~230 KB of pooled partials
    instead of the 6.5 MB node-feature table.
"""

import sys

sys.path.insert(0, "/opt/trn_rl_repo")

import numpy as np
import ml_dtypes

BF16 = ml_dtypes.bfloat16

NC = 8          # cores
H = 4           # attention heads
NEG = 0.2       # leaky relu slope


def make_cfg(n_nodes, n_graphs, k_ch):
    cfg = {}
    cfg["N"] = n_nodes
    cfg["G"] = n_graphs
    assert n_nodes % NC == 0
    cfg["NPART"] = n_nodes // NC
    cfg["LAYERS"] = [(128, 128), (256, 128), (256, 256)]  # (F_out, F_in)
    cfg["K_CH"] = k_ch              # chunks per dst block
    cfg["SLOT"] = 128 * k_ch
    cfg["BN"] = 32                  # node-slots per block
    cfg["KB"] = 8 * k_ch            # chunks per metadata group (8 blocks)
    return cfg


def cfg_layer(cfg, l):
    F, Fin = cfg["LAYERS"][l]
    RC = ((F + 9 + 15) // 16) * 16   # bf16 table row width
    GC = F + 5                       # gathered cols: h + one + s_src
    return F, Fin, RC, GC


FULL_CFG = make_cfg(50000, 128, 4)


def _prep_core(c, src, dst, cfg):
    npart, SLOT, BN = cfg["NPART"], cfg["SLOT"], cfg["BN"]
    m = (dst // npart) == c
    eids = np.flatnonzero(m)
    s_c = src[m]
    dloc = (dst[m] - c * npart).astype(np.int64)
    o = np.argsort(dloc, kind="stable")
    s_c, dloc, eids = s_c[o], dloc[o], eids[o]
    counts = np.bincount(dloc, minlength=npart)
    assert counts.max() <= SLOT
    blocks = []
    base = 0
    while base < npart:
        nv, tot = 0, 0
        while base + nv < npart and nv < BN:
            ce = counts[base + nv]
            if tot + ce > SLOT:
                break
            tot += int(ce)
            nv += 1
        assert nv > 0
        blocks.append((base, nv, tot))
        base += nv
    return {"s": s_c, "dloc": dloc, "blocks": blocks, "eids": eids}


def host_prep(inputs, cfg):
    N, G = cfg["N"], cfg["G"]
    npart = cfg["NPART"]
    K_CH, SLOT, KB, BN = cfg["K_CH"], cfg["SLOT"], cfg["KB"], cfg["BN"]

    x = np.asarray(inputs["x"], np.float32)
    XGC = 136                      # x-row(128) + one + pad
    ei = np.asarray(inputs["edge_index"]).astype(np.int64)
    bi = np.asarray(inputs["batch_index"]).astype(np.int64)
    loops = np.arange(N, dtype=np.int64)
    src = np.concatenate([ei[0], loops])
    dst = np.concatenate([ei[1], loops])

    Ws = [np.asarray(inputs[f"W{l}"], np.float32) for l in range(3)]
    asrcs = [np.asarray(inputs[f"a_src{l}"], np.float32) for l in range(3)]
    adsts = [np.asarray(inputs[f"a_dst{l}"], np.float32) for l in range(3)]
    bs = [np.asarray(inputs[f"b{l}"], np.float32) for l in range(3)]

    waugs = []
    for l in range(3):
        F, Fin, RC, GC = cfg_layer(cfg, l)
        Fo = F // H
        Wa = np.zeros((Fin, RC), np.float32)
        Wa[:, :F] = Ws[l]
        W3 = Ws[l].reshape(Fin, H, Fo)
        Wa[:, F + 1 : F + 5] = np.einsum("fhk,hk->fh", W3, asrcs[l])
        Wa[:, F + 5 : F + 9] = np.einsum("fhk,hk->fh", W3, adsts[l])
        waugs.append(Wa)

    # host-computed layer-0 attention numerators (layer 0 is gather-free)
    h0 = x @ Ws[0]
    h03 = h0.reshape(N, H, 32)
    ss0 = np.einsum("nhf,hf->nh", h03, asrcs[0])
    sd0 = np.einsum("nhf,hf->nh", h03, adsts[0])
    e0 = ss0[src] + sd0[dst]
    e0 = np.where(e0 > 0, e0, NEG * e0)
    ex0_all = np.exp(e0).astype(BF16)           # [Etot, 4]

    cores = [_prep_core(c, src, dst, cfg) for c in range(NC)]
    n_blk = max(len(ci["blocks"]) for ci in cores)
    n_blk = ((n_blk + 7) // 8) * 8
    nch = n_blk * K_CH
    ngrp = nch // KB
    SH = BN * n_blk                  # slot rows per core (32 per block)
    DCH = SH // 128
    cfg["N_BLK"], cfg["NCH"], cfg["NGRP"] = n_blk, nch, ngrp
    cfg["SH_ROWS"], cfg["DENSE_CH"], cfg["TAB_ROWS"] = SH, DCH, NC * SH

    # node -> slot maps (per core); slot = 32*b + (node - base_b)
    slot_of = np.zeros((NC, npart), np.int64)
    for c in range(NC):
        for b, (base, nv, tot) in enumerate(cores[c]["blocks"]):
            slot_of[c, base : base + nv] = BN * b + np.arange(nv)

    def table_row(g):
        r = g // npart
        return (r * SH + slot_of[r, g - r * npart]).astype(np.int32)

    in_maps = []
    all_segs = []
    for c in range(NC):
        ci = cores[c]
        srcslot = np.zeros(nch * 128, np.int32)
        s01 = np.zeros((nch * 128, 32), BF16)
        xg0 = np.zeros((nch * 128, XGC), BF16)
        ex0 = np.ones((nch * 128, 4), BF16)
        pos = 0
        srows = table_row(ci["s"])
        for b, (base, nv, tot) in enumerate(ci["blocks"]):
            sl0 = b * SLOT
            srcslot[sl0 : sl0 + tot] = srows[pos : pos + tot]
            dl = (ci["dloc"][pos : pos + tot] - base).astype(np.int64)
            s01[sl0 + np.arange(tot), dl] = BF16(1.0)
            xg0[sl0 : sl0 + tot, 0:128] = x[ci["s"][pos : pos + tot]]
            xg0[sl0 : sl0 + tot, 128] = BF16(1.0)
            ex0[sl0 : sl0 + tot] = ex0_all[ci["eids"][pos : pos + tot]]
            pos += tot
        # chunk-slot-major regrouping: chunk j of group g holds slots
        # [ (g*KB+j)*128, +128 ); within a block, slot s -> (chunk s//128,
        # partition s%128)
        srcg = np.ascontiguousarray(
            srcslot.reshape(ngrp, KB, 128).transpose(0, 2, 1))
        s01g = np.ascontiguousarray(
            s01.reshape(ngrp, KB, 128, 32).transpose(0, 2, 1, 3)
        ).reshape(ngrp, 128, KB * 32)
        s01Tg = np.ascontiguousarray(
            s01.reshape(ngrp, KB, 128, 32).transpose(0, 3, 1, 2)
        ).reshape(ngrp, 32, KB * 128)
        xg0g = np.ascontiguousarray(
            xg0.reshape(ngrp, KB, 128, XGC).transpose(0, 2, 1, 3)
        ).reshape(ngrp, 128, KB * XGC)
        ex0g = np.ascontiguousarray(
            ex0.reshape(ngrp, KB, 128, 4).transpose(0, 2, 1, 3)
        ).reshape(ngrp, 128, KB * 4)

        # ---- pooling metadata on the slot grid: graph ids per slot,
        # 2-segment max masks (a 128-slot chunk spans <= 2 graphs)
        slot_graph = np.full(SH, 255, np.int64)
        for b, (base, nv, tot) in enumerate(ci["blocks"]):
            slot_graph[BN * b : BN * b + nv] = bi[
                c * npart + base : c * npart + base + nv]
        bic = np.full((128, DCH), 255.0, np.float32)
        wall = np.zeros((128, DCH, 2), np.float32)
        segs = []
        for t in range(DCH):
            gb = slot_graph[t * 128 : (t + 1) * 128]
            real = gb != 255
            if not real.any():
                segs.append((-1, -1))
                continue
            rg = gb[real]
            g1, g2 = int(rg[0]), int(rg[-1])
            bic[:, t] = gb
            wall[:, t, 0] = gb == g1
            if g2 != g1:
                wall[:, t, 1] = gb == g2
            else:
                g2 = -1
            segs.append((g1, g2))
        all_segs.append(segs)

        in_maps.append({
            "xg0g": xg0g,
            "ex0g": ex0g,
            "srcg": srcg,
            "s01g": s01g,
            "s01Tg": s01Tg,
            "waug0": waugs[0],
            "waug1": waugs[1],
            "waug2": waugs[2],
            "bias0": bs[0].reshape(1, 128).T.copy(),
            "bias1": np.stack([bs[1][:128], bs[1][128:]], 1),
            "bic": bic.astype(BF16),
            "wall": wall.reshape(128, DCH * 2).astype(BF16),
            "b2rep": np.broadcast_to(bs[2], (128, 256)).astype(np.float32).copy(),
        })

    merge_meta = {
        "segs": all_segs,
        "cnt": np.bincount(bi, minlength=G).astype(np.float32),
        "Wout": np.asarray(inputs["Wout"], np.float32),
        "bout": np.asarray(inputs["bout"], np.float32),
    }
    return in_maps, merge_meta


def build(cfg):
    from concourse import bass, mybir, tile, bacc
    from concourse.masks import make_identity

    f32 = mybir.dt.float32
    bf16 = mybir.dt.bfloat16
    i32 = mybir.dt.int32

    NCH, NGRP = cfg["NCH"], cfg["NGRP"]
    N_BLK, K_CH, KB, BN = cfg["N_BLK"], cfg["K_CH"], cfg["KB"], cfg["BN"]
    SH = cfg["SH_ROWS"]
    TAB = cfg["TAB_ROWS"]
    DCH = cfg["DENSE_CH"]
    rg = [list(range(NC))]

    nc = bacc.Bacc("TRN2", target_bir_lowering=False, debug=False,
                   num_devices=NC)

    XGC = 136
    xg0g = nc.dram_tensor("xg0g", [NGRP, 128, KB * XGC], bf16,
                          kind="ExternalInput")
    ex0g = nc.dram_tensor("ex0g", [NGRP, 128, KB * 4], bf16,
                          kind="ExternalInput")
    srcg = nc.dram_tensor("srcg", [NGRP, 128, KB], i32, kind="ExternalInput")
    s01g = nc.dram_tensor("s01g", [NGRP, 128, KB * 32], bf16,
                          kind="ExternalInput")
    s01Tg = nc.dram_tensor("s01Tg", [NGRP, 32, KB * 128], bf16,
                           kind="ExternalInput")
    waug_in = []
    for l in range(3):
        F, Fin, RC, GC = cfg_layer(cfg, l)
        waug_in.append(
            nc.dram_tensor(f"waug{l}", [Fin, RC], f32, kind="ExternalInput"))
    bias0 = nc.dram_tensor("bias0", [128, 1], f32, kind="ExternalInput")
    bias1 = nc.dram_tensor("bias1", [128, 2], f32, kind="ExternalInput")
    bic_in = nc.dram_tensor("bic", [128, DCH], bf16, kind="ExternalInput")
    wall_in = nc.dram_tensor("wall", [128, DCH * 2], bf16, kind="ExternalInput")
    b2rep = nc.dram_tensor("b2rep", [128, 256], f32, kind="ExternalInput")

    pool_out = nc.dram_tensor("pool_out", [128, 256 + DCH * 4], f32,
                              kind="ExternalOutput")

    AX = mybir.AxisListType
    OP = mybir.AluOpType
    AF = mybir.ActivationFunctionType

    with tile.TileContext(nc) as tc:
        with tc.tile_pool(name="const", bufs=1) as cpool, \
             tc.tile_pool(name="sb", bufs=3) as sb, \
             tc.tile_pool(name="sb3", bufs=4) as sb3, \
             tc.tile_pool(name="sb4", bufs=8) as sb4, \
             tc.tile_pool(name="psT", bufs=3, space="PSUM") as psT, \
             tc.tile_pool(name="psH", bufs=2, space="PSUM") as psH, \
             tc.tile_pool(name="psB", bufs=2, space="PSUM") as psB, \
             tc.tile_pool(name="psP", bufs=1, space="PSUM") as psP, \
             tc.tile_pool(name="dram", bufs=1, space="DRAM") as dram:

            ident = cpool.tile([128, 128], f32)
            make_identity(nc, ident[:])
            identb = cpool.tile([128, 128], bf16)
            nc.vector.tensor_copy(out=identb[:], in_=ident[:])
            waug_t = []
            for l in range(3):
                F, Fin, RC, GC = cfg_layer(cfg, l)
                tiles = []
                for kt in range(Fin // 128):
                    w = cpool.tile([128, RC], f32, tag=f"waug{l}_{kt}",
                                   name=f"waugt{l}_{kt}")
                    nc.sync.dma_start(
                        out=w[:], in_=waug_in[l][kt * 128 : (kt + 1) * 128, :])
                    tiles.append(w)
                waug_t.append(tiles)
            bias0_t = cpool.tile([128, 1], f32)
            nc.sync.dma_start(out=bias0_t[:], in_=bias0[:, :])
            bias1_t = cpool.tile([128, 2], f32)
            nc.sync.dma_start(out=bias1_t[:], in_=bias1[:, :])

            # pooling constants: graph one-hot selection (device-generated
            # from per-slot graph ids vs an iota), max masks, layer-2 bias
            bict = cpool.tile([128, DCH], bf16)
            nc.sync.dma_start(out=bict[:], in_=bic_in[:, :])
            wallt = cpool.tile([128, DCH * 2], bf16)
            nc.sync.dma_start(out=wallt[:], in_=wall_in[:, :])
            b2t = cpool.tile([128, 256], f32)
            nc.sync.dma_start(out=b2t[:], in_=b2rep[:, :])
            ioi = cpool.tile([128, 128], mybir.dt.int32)
            nc.gpsimd.iota(ioi[:], pattern=[[1, 128]], base=0,
                           channel_multiplier=0)
            iob = cpool.tile([128, 128], bf16)
            nc.vector.tensor_copy(out=iob[:], in_=ioi[:])
            sgt = cpool.tile([128, DCH * 128], bf16)
            nc.vector.tensor_tensor(
                out=sgt[:].rearrange("p (t g) -> p t g", g=128),
                in0=bict[:][:, :, None].to_broadcast([128, DCH, 128]),
                in1=iob[:][:, None, :].to_broadcast([128, DCH, 128]),
                op=OP.is_equal)
            gmst = cpool.tile([128, DCH * 4], f32)

            shard, table, xnext = {}, {}, {}
            for l in (1, 2):
                F, Fin, RC, GC = cfg_layer(cfg, l)
                shard[l] = dram.tile([SH, RC], bf16, tag=f"shard{l}",
                                     name=f"shard{l}")
                table[l] = dram.tile([TAB, RC], bf16, tag=f"table{l}",
                                     name=f"table{l}", addr_space="Shared")
            for l in range(1, 4):
                F = cfg["LAYERS"][l - 1][0]
                Fo = F // H
                xnext[l] = dram.tile([4 * SH, Fo], f32, tag=f"xnext{l}",
                                     name=f"xnext{l}")

            pg = psP.tile([128, 256], f32, tag="pg")

            def emit_dense(l, t):
                # dense chunk t of layer l: needs only xnext[l] slot rows
                # [128t, 128t+128) — i.e. edge blocks 4t..4t+3 of layer l-1 —
                # so it is emitted interleaved into layer l-1's edge-group
                # loop and overlaps with the gather-bound edge phase.
                F, Fin, RC, GC = cfg_layer(cfg, l)
                KT = Fin // 128
                FoP = cfg["LAYERS"][l - 1][0] // H
                hper = 128 // FoP
                lhsts = []
                for kt in range(KT):
                    # x stored head-major [4, SH, FoP]; read
                    # [128 slots, 128 fin] with fin = (h, fo)
                    raw = sb.tile([128, 128], f32, tag="xraw", bufs=6)
                    src_ap = xnext[l][:].rearrange(
                        "(h r) f -> r h f", h=4)[
                        t * 128 : (t + 1) * 128,
                        kt * hper : (kt + 1) * hper, :]
                    nc.sync.dma_start(out=raw[:], in_=src_ap)
                    tp = psT.tile([128, 128], f32, tag="tp")
                    nc.tensor.transpose(out=tp[:], in_=raw[:],
                                        identity=ident[:])
                    xt = sb.tile([128, 128], f32, tag="xt", bufs=6)
                    bt = bias0_t if l == 1 else bias1_t
                    nc.scalar.activation(
                        out=xt[:], in_=tp[:], func=AF.Relu,
                        bias=bt[:, kt : kt + 1], scale=1.0)
                    lhsts.append(xt)
                hp = psH.tile([128, RC], f32, tag="hp")
                for kt in range(KT):
                    nc.tensor.matmul(out=hp[:], lhsT=lhsts[kt][:],
                                     rhs=waug_t[l][kt][:],
                                     start=(kt == 0), stop=(kt == KT - 1))
                row = sb.tile([128, RC], bf16, tag="row")
                nc.vector.tensor_copy(out=row[:], in_=hp[:])
                nc.vector.memset(row[:, F : F + 1], 1.0)
                nc.sync.dma_start(
                    out=shard[l][t * 128 : (t + 1) * 128, :], in_=row[:])

            def emit_pool(t):
                # readout pooling chunk t: needs xnext[3] slot rows of edge
                # blocks 4t..4t+3 of layer 2 — interleaved into layer 2's
                # edge-group loop.
                if cfg.get("skip_pool"):
                    return
                praw = sb.tile([128, 256], f32, tag="praw")
                nc.sync.dma_start(
                    out=praw[:],
                    in_=xnext[3][:].rearrange("(h r) f -> r h f", h=4)[
                        t * 128 : (t + 1) * 128, :, :])
                phs = sb3.tile([128, 256], f32, tag="phs")
                nc.vector.tensor_tensor(out=phs[:], in0=praw[:], in1=b2t[:],
                                        op=OP.add)
                phb = sb3.tile([128, 256], bf16, tag="phb")
                nc.vector.tensor_scalar(out=phb[:], in0=phs[:], scalar1=0.0,
                                        scalar2=None, op0=OP.max)
                nc.tensor.matmul(out=pg[:], lhsT=sgt[:, t * 128 : (t + 1) * 128],
                                 rhs=phb[:], start=(t == 0),
                                 stop=(t == DCH - 1))
                if cfg.get("skip_pool_max"):
                    return
                pmsk = sb3.tile([128, 2 * 256], bf16, tag="pmsk")
                nc.vector.tensor_tensor(
                    out=pmsk[:].rearrange("p (s f) -> p s f", f=256),
                    in0=phb[:][:, None, :].to_broadcast([128, 2, 256]),
                    in1=wallt[:, 2 * t : 2 * t + 2][:, :, None].to_broadcast(
                        [128, 2, 256]),
                    op=OP.mult)
                for half in range(2):
                    pttm = psH.tile([128, 256], bf16, tag="hp", name="pttm")
                    for k in range(2):
                        kk = half * 2 + k
                        nc.tensor.transpose(
                            out=pttm[:, k * 128 : (k + 1) * 128],
                            in_=pmsk[:, kk * 128 : (kk + 1) * 128],
                            identity=identb[:])
                    nc.vector.tensor_reduce(
                        out=gmst[:, 4 * t + 2 * half : 4 * t + 2 * half + 2],
                        in_=pttm[:].rearrange("p (k f) -> p k f", f=128),
                        axis=AX.X, op=OP.max)

            emitted_dense = {1: False, 2: False}
            for l in range(3):
                F, Fin, RC, GC = cfg_layer(cfg, l)
                Fo = F // H

                # ---- dense phase fallback (normally interleaved into the
                # previous layer's edge loop; layer 0 is host-streamed) ----
                if l > 0 and not emitted_dense[l]:
                    for t in range(DCH):
                        emit_dense(l, t)

                # ---- halo exchange ----
                if l > 0 and not cfg.get("skip_ag"):
                    nc.gpsimd.collective_compute(
                        "AllGather", OP.bypass, replica_groups=rg,
                        ins=[shard[l][:, :]], outs=[table[l][:, :]])

                # ---- edge phase ----
                for g in range(0 if cfg.get("skip_edges") else NGRP):
                    if l == 0:
                        xgt = sb.tile([128, KB * XGC], bf16, tag="xgt")
                        nc.sync.dma_start(out=xgt[:], in_=xg0g[g, :, :])
                        ext = sb.tile([128, KB * 4], bf16, tag="ext")
                        nc.sync.dma_start(out=ext[:], in_=ex0g[g, :, :])
                        ext3 = ext[:].rearrange("p (k c) -> p k c", c=4)
                        xgt3 = xgt[:].rearrange("p (k c) -> p k c", c=XGC)
                    else:
                        sidx = sb.tile([128, KB], i32, tag="sidx")
                        nc.sync.dma_start(out=sidx[:], in_=srcg[g, :, :])
                        s01Tt = sb.tile([32, KB * 128], bf16, tag="s01Tt")
                        nc.sync.dma_start(out=s01Tt[:], in_=s01Tg[g, :, :])
                    s01t = sb.tile([128, KB * 32], bf16, tag="s01t")
                    nc.sync.dma_start(out=s01t[:], in_=s01g[g, :, :])
                    s013 = s01t[:].rearrange("p (k d) -> p k d", d=32)

                    if l > 0:
                        # all 8 blocks' s_dst rows in one strided DMA:
                        # sdh_g[d, b, h] <- shard[32*(8g+b)+d, F+5+h]
                        sdh_g = sb3.tile([32, 8 * 4], bf16, tag="sdhg")
                        nc.sync.dma_start(
                            out=sdh_g[:].rearrange("d (b h) -> d b h", h=4),
                            in_=shard[l][:].rearrange(
                                "(q d) c -> d q c", d=BN)[
                                :, g * 8 : g * 8 + 8, F + 5 : F + 9])
                    # per-group scatter staging: block bb writes cols bb*Fo
                    xnd_g = sb3.tile([128, 8 * Fo], f32, tag="xndg")
                    for bb in range(8):          # blocks within the group
                        blk = g * 8 + bb
                        j0 = bb * K_CH
                        stile = sb3.tile([128, K_CH * 128], bf16, tag="stile")
                        st4 = stile[:].rearrange("p (k h d) -> p k h d",
                                                 h=H, d=32)
                        if l == 0:
                            nc.vector.tensor_tensor(
                                out=st4,
                                in0=s013[:, j0 : j0 + K_CH, :][
                                    :, :, None, :].to_broadcast(
                                    [128, K_CH, H, 32]),
                                in1=ext3[:, j0 : j0 + K_CH, :][
                                    :, :, :, None].to_broadcast(
                                    [128, K_CH, H, 32]),
                                op=OP.mult)
                        else:
                            # per-edge s_dst via s01T^T @ sdh (sdh loaded
                            # per group from the local shard, static base)
                            sdh = sdh_g[:, bb * 4 : bb * 4 + 4]
                            gat = sb4.tile([128, K_CH * GC], bf16, tag="gat")
                            g4 = gat[:].rearrange("p (k c) -> p k c", c=GC)
                            for k in range(K_CH):
                                nc.gpsimd.indirect_dma_start(
                                    out=g4[:, k, :], out_offset=None,
                                    in_=table[l][:],
                                    in_offset=bass.IndirectOffsetOnAxis(
                                        ap=sidx[:, j0 + k : j0 + k + 1],
                                        axis=0))
                            sdx = psB.tile([128, K_CH * 4], f32, tag="pblk", name="sdx")
                            for k in range(K_CH):
                                nc.tensor.matmul(
                                    out=sdx[:, k * 4 : (k + 1) * 4],
                                    lhsT=s01Tt[:, (j0 + k) * 128 :
                                               (j0 + k + 1) * 128],
                                    rhs=sdh, start=True, stop=True)
                            esca = sb3.tile([128, K_CH * 4], f32, tag="esca")
                            nc.vector.tensor_tensor(
                                out=esca[:].rearrange("p (k h) -> p k h", h=4),
                                in0=sdx[:].rearrange("p (k h) -> p k h", h=4),
                                in1=g4[:, :, F + 1 : F + 5],
                                op=OP.add)
                            esc2 = sb3.tile([128, K_CH * 4], f32, tag="esc2")
                            nc.vector.scalar_tensor_tensor(
                                out=esc2[:], in0=esca[:], scalar=NEG,
                                in1=esca[:], op0=OP.mult, op1=OP.max)
                            exa = sb3.tile([128, K_CH * 4], f32, tag="exa")
                            nc.scalar.activation(out=exa[:], in_=esc2[:],
                                                 func=AF.Exp)
                            nc.vector.tensor_tensor(
                                out=st4,
                                in0=s013[:, j0 : j0 + K_CH, :][
                                    :, :, None, :].to_broadcast(
                                    [128, K_CH, H, 32]),
                                in1=exa[:].rearrange("p (k h) -> p k h", h=4)[
                                    :, :, :, None].to_broadcast(
                                    [128, K_CH, H, 32]),
                                op=OP.mult)
                        NAGG = 130 if l == 0 else F + 1
                        pblk = psB.tile([128, NAGG], f32, tag="pblk")
                        for k in range(K_CH):
                            rhs_ap = (xgt3[:, j0 + k, 0:129] if l == 0
                                      else g4[:, k, 0 : F + 1])
                            nc.tensor.matmul(
                                out=pblk[:, 0 : (129 if l == 0 else F + 1)],
                                lhsT=stile[:, k * 128 : (k + 1) * 128],
                                rhs=rhs_ap,
                                start=(k == 0), stop=(k == K_CH - 1))
                        if l == 0:
                            # xagg -> transpose -> @W0
                            xac = sb3.tile([128, 129], f32, tag="xac")
                            nc.vector.tensor_copy(out=xac[:],
                                                  in_=pblk[:, 0:129])
                            tps = psT.tile([128, 128], f32, tag="tp",
                                           name="tps")
                            nc.tensor.transpose(out=tps[:],
                                                in_=xac[:, 0:128],
                                                identity=ident[:])
                            xaT = sb3.tile([128, 128], f32, tag="xaT")
                            nc.scalar.activation(out=xaT[:], in_=tps[:],
                                                 func=AF.Copy)
                            po = psH.tile([128, 128], f32, tag="hp",
                                          name="po")
                            nc.tensor.matmul(
                                out=po[:], lhsT=xaT[:],
                                rhs=waug_t[0][0][:, 0:128],
                                start=True, stop=True)
                            den_src = xac[:, 128:129]
                            norm_src = po[:]
                        else:
                            den_src = pblk[:, F : F + 1]
                            norm_src = pblk[:, 0:F]
                        den = sb3.tile([128, 1], f32, tag="den")
                        nc.vector.tensor_scalar(
                            out=den[:], in0=den_src,
                            scalar1=1e-20, scalar2=None, op0=OP.add)
                        rec = sb3.tile([128, 1], f32, tag="rec")
                        nc.vector.reciprocal(out=rec[:], in_=den[:])
                        xn = sb3.tile([128, F], f32, tag="xn")
                        nc.scalar.activation(
                            out=xn[:, 0:F], in_=norm_src, func=AF.Copy,
                            scale=rec[:, 0:1])
                        for hh in range(H):
                            nc.vector.tensor_copy(
                                out=xnd_g[hh * 32 : (hh + 1) * 32,
                                          bb * Fo : (bb + 1) * Fo],
                                in_=xn[hh * 32 : (hh + 1) * 32,
                                       hh * Fo : (hh + 1) * Fo])
                    # group scatter: one strided DMA per head section
                    # (static base): rows 32*(8g+b)+d of head hh get
                    # xnd_g[hh*32+d, b*Fo:(b+1)*Fo]
                    for hh in range(H):
                        nc.sync.dma_start(
                            out=xnext[l + 1][:].rearrange(
                                "(h q d) f -> h d q f", h=4, d=BN)[
                                hh, :, g * 8 : g * 8 + 8, :],
                            in_=xnd_g[hh * 32 : (hh + 1) * 32, :].rearrange(
                                "d (b f) -> d b f", f=Fo))
                    # this group completed blocks 8g..8g+7 = xnext[l+1] slot
                    # chunks 2g, 2g+1: emit the consumers now so they overlap
                    # with the remaining (gather-bound) edge groups
                    if l < 2:
                        emitted_dense[l + 1] = True
                        emit_dense(l + 1, 2 * g)
                        emit_dense(l + 1, 2 * g + 1)
                    else:
                        emit_pool(2 * g)
                        emit_pool(2 * g + 1)

            # ---- on-device readout pooling: chunks were emitted
            # interleaved into layer 2's edge loop; only the final
            # copies/stores remain here ----
            if not cfg.get("skip_pool"):
                pmean = sb3.tile([128, 256], f32, tag="pmean")
                nc.vector.tensor_copy(out=pmean[:], in_=pg[:])
                nc.sync.dma_start(out=pool_out[:, 0:256], in_=pmean[:])
                if not cfg.get("skip_pool_max"):
                    nc.sync.dma_start(out=pool_out[:, 256 : 256 + DCH * 4],
                                      in_=gmst[:])

    nc.compile()
    return nc


def host_merge(results, merge_meta, cfg):
    G = cfg["G"]
    DCH = cfg["DENSE_CH"]
    segs = merge_meta["segs"]
    msum = np.zeros((G, 256), np.float32)
    gmax = np.full((G, 256), -np.inf, np.float32)
    for c in range(NC):
        out = np.asarray(results[c]["pool_out"], np.float32)
        msum += out[:, 0:256]
        mx = out[:, 256:].reshape(128, DCH, 2, 2)
        for t in range(DCH):
            for s in range(2):
                g = segs[c][t][s]
                if g < 0:
                    continue
                np.maximum(gmax[g, 0:128], mx[:, t, s, 0], out=gmax[g, 0:128])
                np.maximum(gmax[g, 128:256], mx[:, t, s, 1],
                           out=gmax[g, 128:256])
    gmean = msum / np.maximum(merge_meta["cnt"], 1.0)[:, None]
    pooled = np.concatenate([gmean, gmax], axis=1)
    return pooled @ merge_meta["Wout"] + merge_meta["bout"]


_CACHE = {}


def _get_compiled(inputs, cfg):
    in_maps, merge_meta = host_prep(inputs, cfg)
    key = (cfg["N"], cfg["N_BLK"])
    if key not in _CACHE:
        _CACHE[key] = build(cfg)
    return _CACHE[key], in_maps, merge_meta


def run(inputs, cfg, trace=False):
    from concourse.bass_utils import run_bass_kernel_spmd

    nc, in_maps, merge_meta = _get_compiled(inputs, cfg)
    r = run_bass_kernel_spmd(nc, in_maps, core_ids=list(range(NC)))
    out = host_merge(r.results, merge_meta, cfg)
    return out.astype(np.float32), r


def kernel(**inputs):
    cfg = dict(FULL_CFG)
    out, _ = run(inputs, cfg)
    return out
